# revision 46
# baseline (speedup 1.0000x reference)
"""nn_Attention_19121194402320 on 8 TRN2 NeuronCores (raw Bass, bf16/fp8).

The reference module is

    k = (key @ Wk.T).reshape(B, H, S, D)       # RAW reshape
    q, v analogously
    attn = softmax(q @ k.T, axis=-1)
    out  = einsum('bnqk,bnvd->bnqd', attn, v)  # NOTE the 'k' vs 'v' labels
    out.transpose(0,2,1,3).reshape(B, S, E)

The second einsum's contraction labels differ ('k' in the first operand,
'v' in the second), so einsum sums each independently:

    out[b,n,q,d] = (sum_k attn[b,n,q,k]) * (sum_v v[b,n,v,d])
                 = sum_v v[b,n,v,d]          (softmax rows sum to 1)

i.e. the output is the per-head column-sum of the V projection broadcast
over every query position; query/key/Wq/Wk do not affect it (verified to
7e-7 against the jax reference).

Math: raw-reshape head h of Y = value@Wv.T is the contiguous flat chunk
Y[b].flat[h*65536:(h+1)*65536].reshape(1024, 64); chunk g = 12s + c maps
to Y[s, 64c:64c+64].  With U[s, c*12+h] the 0/1 indicator that chunk
12s+c belongs to head h:

    step1  Z.T[e, u]  = sum_s Xv[s, e] U[s, u]        (24 mm, xv stationary)
    step2  G.T[j, u]  = sum_e Wv.T[e, j] Z.T[e, u]    (18 mm, wv stationary)
    out[h*64+d]       = sum_c G.T[64c+d, c*12+h]

Sharding: core = (batch, e-half).  Each core contracts HALF the e
dimension (384 of 768): xv = value[b][:, eh*384:], wv = Wv.T[eh*384:,:],
and produces a partial [64, 24] tile; the host sums the two partials per
batch (the contraction is linear, so no cross-core comm).  This halves
the dominant Wv load vs a head-split (590KB vs 1.18MB/core).

Computing G TRANSPOSED puts the needed diagonal blocks at PSUM partition
bases 64c (multiples of 32), so the DVE reads them directly — no
SBUF->SBUF gather DMAs.  Step2 only computes the 24 u-columns per
j-chunk that the extraction uses (rhs = zt[:, eg, 24m:24m+24]), so its
18 matmuls run at the 60ns LDWEIGHTS-pipelined floor.  The c-sum is two
strided tensor_reduces (pgt cols 24m+h on partitions 0:64, 24m+12+h on
64:128 — a fused on-device add of the two raced the DVE pipeline, so
the host adds the 2x12 partial columns instead).  All 1024 output rows
are identical: host transposes + tiles the summed [64, 12] tile.

The mask is exact in fp8 (0/1 values; bf16 x fp8 matmul verified exact
on HW), halving its footprint.  Inputs stream as 4 large DMAs with one
contiguous 1.1-4.6KB descriptor per partition (host pre-permutes wv/um
rows).  Total input 1.52MB/core vs 2.04MB for the head-split version.
All accumulation fp32 in PSUM; 4 PSUM banks (groups are sequential per
bank — interleaved groups in one bank break on HW, verified).

Engine plan:
  sync   : xv slot 0-3 load (1 DMA); final [64,24] output store
  scalar : um + xv slot 4-7 + wv loads; zt cast eg=1
           (ACT_TABLE_LOAD hides under the input stream)
  PE     : step1 Z.T (24 mm, 3 psum banks); step2 G.T (18 mm, 1 bank)
  DVE    : zt casts eg=0,2 (fp32->bf16), 2 strided reduces
"""

from contextlib import ExitStack

import ml_dtypes
import numpy as np

import concourse.bass as bass
from concourse import bacc, mybir
from concourse.bass_utils import run_bass_kernel_spmd

B, S, E, H, D = 4, 1024, 768, 12, 64
EH = E // 2          # e-columns per core (384)
EC = EH // 128       # 3 e-chunks per core
NSL = 8              # s-slots of 128 rows (full 1024)
JC = E // 128        # 6 j-chunks
NU = 144             # mask columns, index c*12 + h
TW = 24              # u-columns touched per j-chunk (2 c-parities x 12 heads)
FP = mybir.dt.float32
BF = mybir.dt.bfloat16
F8 = mybir.dt.float8e4

_CACHE = {}


def _umask() -> np.ndarray:
    """U[s, c*12+h] = 1 iff chunk 12*s+c belongs to head h."""
    U = np.zeros((S, NU), np.float32)
    for c in range(12):
        for h in range(H):
            lo = max(0, (1024 * h - c + 11) // 12)
            hi = min(S, (1024 * (h + 1) - c + 11) // 12)
            U[lo:hi, c * H + h] = 1.0
    return U


def _build_nc():
    # Bass.__init__ unconditionally emits 4 const-tile memsets (gpsimd) and a
    # full all-engine barrier before user code; this kernel uses neither,
    # so suppress them during construction to shave NEFF startup time.
    _memset = bass.BassGpSimd.memset
    _barrier = bass.Bass.all_engine_barrier
    bass.BassGpSimd.memset = lambda self, ap, c: None
    bass.Bass.all_engine_barrier = lambda self, **kw: None
    try:
        nc = bacc.Bacc("TRN2", target_bir_lowering=False, debug=False)
    finally:
        bass.BassGpSimd.memset = _memset
        bass.Bass.all_engine_barrier = _barrier

    # s-mapping shared by xv/um: slot = 4k+r, partition t <-> s = 512k+4t+r
    # (one contiguous 3072B xv descriptor per partition per DMA; ring
    # descriptor fetch is serial with the transfer, so few/large
    # descriptors shorten the stream).  Queue split is lopsided (SP: xv
    # slots 0-3; Act: um + slots 4-7 + wv): concurrent queues share the
    # 16 rings per-descriptor round-robin (~130GB/s each vs ~310 solo),
    # so the small SP stream drains early and Act then runs alone; wv -
    # whose post-arrival work (18 small matmuls) is shortest - lands
    # last, just before it is needed.
    # xum row s = [xv row s (384 bf16) | mask row U[s] (144 fp8, packed
    # into 72 bf16 slots)]: the mask rides inside the xv DMAs - zero
    # extra descriptors or issues; the matmul reads it via a bitcast view.
    xum_d = nc.dram_tensor("xum", [S, EH + NU // 2], BF,
                           kind="ExternalInput").ap()
    # wv rows host-permuted: wv_d[3t + eg] = Wv.T[eh*384 + eg*128 + t]
    wv_d = nc.dram_tensor("wv", [EH, E], BF, kind="ExternalInput").ap()
    # out_d[d, c_parity*12 + h]: partial sums; host adds parities + e-halves,
    # transposes to the 768-wide row and tiles it over all 1024 s positions.
    out_d = nc.dram_tensor("out", [64, 2 * H], FP, kind="ExternalOutput").ap()

    comb_sb = nc.alloc_sbuf_tensor("comb_sb", [128, NSL, EH + NU // 2],
                                   BF).ap()
    wv_sb = nc.alloc_sbuf_tensor("wv_sb", [128, EC, E], BF).ap()
    zt_sb = nc.alloc_sbuf_tensor("zt_sb", [128, EC, NU], BF).ap()
    osum = nc.alloc_sbuf_tensor("osum", [64, 2, H], FP).ap()

    with ExitStack() as ctx:
        pz = [ctx.enter_context(nc.psum_tensor(f"pz{i}", [128, NU], FP))
              for i in range(EC)]
        pgt_h = ctx.enter_context(nc.psum_tensor("pgt", [128, JC * TW], FP))
        pgt = pgt_h.ap()
        dxv = [ctx.enter_context(nc.semaphore(f"dxv{i}")) for i in range(2)]
        dwv = ctx.enter_context(nc.semaphore("dwv"))
        dres = ctx.enter_context(nc.semaphore("dres"))
        dout = ctx.enter_context(nc.semaphore("dout"))
        pe_sem = ctx.enter_context(nc.semaphore("pe_sem"))
        dveA = ctx.enter_context(nc.semaphore("dveA"))
        dveB = ctx.enter_context(nc.semaphore("dveB"))
        block = ctx.enter_context(nc.Block())

        def xv_dma(eng, k):
            eng.dma_start(
                comb_sb[:, 4 * k:4 * k + 4, :],
                xum_d[512 * k:512 * (k + 1), :].rearrange(
                    "(t r) e -> t r e", t=128)
            ).then_inc(dxv[k], 16)

        @block.sync
        def _(sync: bass.BassEngine):
            xv_dma(sync, 0)
            sync.wait_ge(dres, 1)
            sync.dma_start(out_d, osum.rearrange("p a b -> p (a b)")
                           ).then_inc(dout, 16)
            sync.wait_ge(dout, 16)

        @block.scalar
        def _(scalar: bass.BassEngine):
            xv_dma(scalar, 1)
            scalar.dma_start(wv_sb, wv_d.rearrange("(t eg) j -> t eg j", t=128)
                             ).then_inc(dwv, 16)
            # eg=1 cast off the DVE's critical path; the implicit
            # ACT_TABLE_LOAD runs right after the issues, hidden under the
            # input stream (gpsimd cannot read PSUM)
            scalar.wait_ge(pe_sem, 2)
            nc.scalar.copy(zt_sb[:, 1, :], pz[1][:, 0:NU]).then_inc(dveB)

        @block.tensor
        def _(tensor: bass.BassEngine):
            for sl in range(NSL):
                if sl % 4 == 0:
                    tensor.wait_ge(dxv[sl // 4], 16)
                um_ap = comb_sb[:, sl, EH:EH + NU // 2].bitcast(F8)
                for eg in range(EC):
                    mm = nc.tensor.matmul(pz[eg][:, 0:NU],
                                          comb_sb[:, sl, eg * 128:(eg + 1) * 128],
                                          um_ap,
                                          start=(sl == 0), stop=(sl == NSL - 1))
                    if sl == NSL - 1:
                        mm.then_inc(pe_sem)
            # step2: G.T restricted to the 24 used u-cols per j-chunk;
            # m-outer keeps the packed-bank psum groups sequential
            for m in range(JC):
                for eg in range(EC):
                    if m == 0:
                        tensor.wait_ge(dveB if eg == 1 else dveA,
                                       1 if eg < 2 else 2)
                        tensor.wait_ge(dwv, 16)
                    mm = nc.tensor.matmul(pgt[:, m * TW:(m + 1) * TW],
                                          wv_sb[:, eg, m * 128:(m + 1) * 128],
                                          zt_sb[:, eg, m * TW:(m + 1) * TW],
                                          start=(eg == 0), stop=(eg == EC - 1))
            mm.then_inc(pe_sem)

        @block.vector
        def _(vector: bass.BassEngine):
            vector.wait_ge(pe_sem, 1)
            nc.vector.tensor_copy(zt_sb[:, 0, :], pz[0][:, 0:NU]).then_inc(dveA)
            vector.wait_ge(pe_sem, 3)
            nc.vector.tensor_copy(zt_sb[:, 2, :], pz[2][:, 0:NU]).then_inc(dveA)
            vector.wait_ge(pe_sem, 4)
            # diagonal-block c-sum straight out of PSUM: col 24m + h on
            # partitions 0:64 (even c = 2m), col 24m + 12 + h on 64:128
            # (odd c = 2m+1); reduce over m (innermost, stride 24)
            row = JC * TW
            half0 = bass.AP(pgt_h, 0, [[row, 64], [1, H], [TW, JC]])
            half1 = bass.AP(pgt_h, 64 * row + H, [[row, 64], [1, H], [TW, JC]])
            nc.vector.tensor_reduce(osum[:, 0, :], half0,
                                    mybir.AxisListType.X, mybir.AluOpType.add)
            nc.vector.tensor_reduce(osum[:, 1, :], half1,
                                    mybir.AxisListType.X, mybir.AluOpType.add
                                    ).then_inc(dres)

    nc.compile()
    return nc


def _get_nc():
    if "nc" not in _CACHE:
        _CACHE["nc"] = _build_nc()
    return _CACHE["nc"]


def _in_maps(inputs):
    v = np.ascontiguousarray(np.asarray(inputs["value"], dtype=np.float32))
    wvT = np.ascontiguousarray(np.asarray(inputs["Wv"], np.float32).T)
    um8 = np.ascontiguousarray(
        _umask().astype(ml_dtypes.float8_e4m3)).view(np.uint8)
    maps = []
    for c in range(8):
        b, eh = c // 2, c % 2
        xv = np.ascontiguousarray(v[b][:, eh * EH:(eh + 1) * EH]
                                  ).astype(ml_dtypes.bfloat16)
        xum = np.ascontiguousarray(
            np.concatenate([xv.view(np.uint8), um8], axis=1)
        ).view(ml_dtypes.bfloat16)
        wvh = wvT[eh * EH:(eh + 1) * EH, :]
        # wv_d[3t + eg] = wvh[eg*128 + t]
        wvp = np.ascontiguousarray(
            wvh.reshape(EC, 128, E).transpose(1, 0, 2).reshape(EH, E)
        ).astype(ml_dtypes.bfloat16)
        maps.append({"xum": xum, "wv": wvp})
    return maps


def _assemble(results):
    out = np.empty((B, S, E), np.float32)
    for b in range(B):
        t = results[2 * b]["out"] + results[2 * b + 1]["out"]
        row = np.ascontiguousarray((t[:, 0:H] + t[:, H:]).T).reshape(1, E)
        out[b] = np.broadcast_to(row, (S, E))
    return out


def run(inputs, trace=False, **kw):
    """Run on hardware; returns (full_output, BassKernelResults)."""
    nc = _get_nc()
    res = run_bass_kernel_spmd(nc, _in_maps(inputs), core_ids=list(range(8)),
                               trace=trace, **kw)
    return _assemble(res.results), res


def kernel(**inputs) -> np.ndarray:
    out, _ = run(inputs)
    return out


# revision 47
# speedup vs baseline: 1.0021x; 1.0021x over previous
"""nn_Attention_19121194402320 on 8 TRN2 NeuronCores (raw Bass, bf16/fp8).

The reference module is

    k = (key @ Wk.T).reshape(B, H, S, D)       # RAW reshape
    q, v analogously
    attn = softmax(q @ k.T, axis=-1)
    out  = einsum('bnqk,bnvd->bnqd', attn, v)  # NOTE the 'k' vs 'v' labels
    out.transpose(0,2,1,3).reshape(B, S, E)

The second einsum's contraction labels differ ('k' in the first operand,
'v' in the second), so einsum sums each independently:

    out[b,n,q,d] = (sum_k attn[b,n,q,k]) * (sum_v v[b,n,v,d])
                 = sum_v v[b,n,v,d]          (softmax rows sum to 1)

i.e. the output is the per-head column-sum of the V projection broadcast
over every query position; query/key/Wq/Wk do not affect it (verified to
7e-7 against the jax reference).

Math: raw-reshape head h of Y = value@Wv.T is the contiguous flat chunk
Y[b].flat[h*65536:(h+1)*65536].reshape(1024, 64); chunk g = 12s + c maps
to Y[s, 64c:64c+64].  With U[s, c*12+h] the 0/1 indicator that chunk
12s+c belongs to head h:

    step1  Z.T[e, u]  = sum_s Xv[s, e] U[s, u]        (24 mm, xv stationary)
    step2  G.T[j, u]  = sum_e Wv.T[e, j] Z.T[e, u]    (18 mm, wv stationary)
    out[h*64+d]       = sum_c G.T[64c+d, c*12+h]

Sharding: core = (batch, e-half).  Each core contracts HALF the e
dimension (384 of 768): xv = value[b][:, eh*384:], wv = Wv.T[eh*384:,:],
and produces a partial [64, 24] tile; the host sums the two partials per
batch (the contraction is linear, so no cross-core comm).  This halves
the dominant Wv load vs a head-split (590KB vs 1.18MB/core).

Computing G TRANSPOSED puts the needed diagonal blocks at PSUM partition
bases 64c (multiples of 32), so the DVE reads them directly — no
SBUF->SBUF gather DMAs.  Step2 only computes the 24 u-columns per
j-chunk that the extraction uses (rhs = zt[:, eg, 24m:24m+24]), so its
18 matmuls run at the 60ns LDWEIGHTS-pipelined floor.  The c-sum is two
strided tensor_reduces (pgt cols 24m+h on partitions 0:64, 24m+12+h on
64:128 — a fused on-device add of the two raced the DVE pipeline, so
the host adds the 2x12 partial columns instead).  All 1024 output rows
are identical: host transposes + tiles the summed [64, 12] tile.

The mask is exact in fp8 (0/1 values; bf16 x fp8 matmul verified exact
on HW) and rides packed inside the xv rows (912B combined rows, read
back via a bitcast AP view), so inputs stream as just 3 large DMAs with
one contiguous 3.6-4.6KB descriptor per partition (ring descriptor
fetch is serial with the transfer, so descriptor count is stream time).
Total input 1.52MB/core vs 2.04MB for the head-split version.
All accumulation fp32 in PSUM; 4 PSUM banks (groups are sequential per
bank — interleaved groups in one bank break on HW, verified).

Engine plan:
  sync   : xv+mask slot 0-3 load (1 DMA); final [64,24] output store
  scalar : xv+mask slot 4-7 + wv loads; zt cast eg=1
           (ACT_TABLE_LOAD hides under the input stream)
  PE     : step1 Z.T (24 mm, 3 psum banks); step2 G.T (18 mm, 1 bank)
  DVE    : zt casts eg=0,2 (fp32->bf16), 2 strided reduces
"""

from contextlib import ExitStack

import ml_dtypes
import numpy as np

import concourse.bass as bass
from concourse import bacc, mybir
from concourse.bass_utils import run_bass_kernel_spmd

B, S, E, H, D = 4, 1024, 768, 12, 64
EH = E // 2          # e-columns per core (384)
EC = EH // 128       # 3 e-chunks per core
NSL = 8              # s-slots of 128 rows (full 1024)
JC = E // 128        # 6 j-chunks
NU = 144             # mask columns, index c*12 + h
TW = 24              # u-columns touched per j-chunk (2 c-parities x 12 heads)
FP = mybir.dt.float32
BF = mybir.dt.bfloat16
F8 = mybir.dt.float8e4

_CACHE = {}


def _umask() -> np.ndarray:
    """U[s, c*12+h] = 1 iff chunk 12*s+c belongs to head h."""
    U = np.zeros((S, NU), np.float32)
    for c in range(12):
        for h in range(H):
            lo = max(0, (1024 * h - c + 11) // 12)
            hi = min(S, (1024 * (h + 1) - c + 11) // 12)
            U[lo:hi, c * H + h] = 1.0
    return U


def _build_nc():
    # Bass.__init__ unconditionally emits 4 const-tile memsets (gpsimd) and a
    # full all-engine barrier before user code; this kernel uses neither,
    # so suppress them during construction to shave NEFF startup time.
    _memset = bass.BassGpSimd.memset
    _barrier = bass.Bass.all_engine_barrier
    bass.BassGpSimd.memset = lambda self, ap, c: None
    bass.Bass.all_engine_barrier = lambda self, **kw: None
    try:
        nc = bacc.Bacc("TRN2", target_bir_lowering=False, debug=False)
    finally:
        bass.BassGpSimd.memset = _memset
        bass.Bass.all_engine_barrier = _barrier

    # s-mapping: slot = 4k+r, partition t <-> s = 512k+4t+r (one
    # contiguous 3648B descriptor per partition per DMA).  Queue split is
    # lopsided (SP: slots 0-3; Act: slots 4-7 + wv): concurrent queues
    # share the 16 rings per-descriptor round-robin (~130GB/s each vs
    # ~310 solo), so the small SP stream drains early and Act then runs
    # alone; wv - whose post-arrival work (18 small matmuls) is shortest
    # - lands last, just before it is needed.
    # xum row s = [xv row s (384 bf16) | mask row U[s] (144 fp8, packed
    # into 72 bf16 slots)]: the mask rides inside the xv DMAs - zero
    # extra descriptors or issues; the matmul reads it via a bitcast view.
    xum_d = nc.dram_tensor("xum", [S, EH + NU // 2], BF,
                           kind="ExternalInput").ap()
    # wv rows host-permuted: wv_d[3t + eg] = Wv.T[eh*384 + eg*128 + t]
    wv_d = nc.dram_tensor("wv", [EH, E], BF, kind="ExternalInput").ap()
    # out_d[d, c_parity*12 + h]: partial sums; host adds parities + e-halves,
    # transposes to the 768-wide row and tiles it over all 1024 s positions.
    out_d = nc.dram_tensor("out", [64, 2 * H], FP, kind="ExternalOutput").ap()

    comb_sb = nc.alloc_sbuf_tensor("comb_sb", [128, NSL, EH + NU // 2],
                                   BF).ap()
    wv_sb = nc.alloc_sbuf_tensor("wv_sb", [128, EC, E], BF).ap()
    zt_sb = nc.alloc_sbuf_tensor("zt_sb", [128, EC, NU], BF).ap()
    osum = nc.alloc_sbuf_tensor("osum", [64, 2, H], FP).ap()

    with ExitStack() as ctx:
        pz = [ctx.enter_context(nc.psum_tensor(f"pz{i}", [128, NU], FP))
              for i in range(EC)]
        pgt_h = ctx.enter_context(nc.psum_tensor("pgt", [128, JC * TW], FP))
        pgt = pgt_h.ap()
        dxv = [ctx.enter_context(nc.semaphore(f"dxv{i}")) for i in range(2)]
        dwv = ctx.enter_context(nc.semaphore("dwv"))
        dres = ctx.enter_context(nc.semaphore("dres"))
        dout = ctx.enter_context(nc.semaphore("dout"))
        pe_sem = ctx.enter_context(nc.semaphore("pe_sem"))
        dveA = ctx.enter_context(nc.semaphore("dveA"))
        dveB = ctx.enter_context(nc.semaphore("dveB"))
        block = ctx.enter_context(nc.Block())

        def xv_dma(eng, k):
            eng.dma_start(
                comb_sb[:, 4 * k:4 * k + 4, :],
                xum_d[512 * k:512 * (k + 1), :].rearrange(
                    "(t r) e -> t r e", t=128)
            ).then_inc(dxv[k], 16)

        @block.sync
        def _(sync: bass.BassEngine):
            xv_dma(sync, 0)
            sync.wait_ge(dres, 1)
            sync.dma_start(out_d, osum.rearrange("p a b -> p (a b)")
                           ).then_inc(dout, 16)
            sync.wait_ge(dout, 16)

        @block.scalar
        def _(scalar: bass.BassEngine):
            xv_dma(scalar, 1)
            scalar.dma_start(wv_sb, wv_d.rearrange("(t eg) j -> t eg j", t=128)
                             ).then_inc(dwv, 16)
            # eg=1 cast off the DVE's critical path; the implicit
            # ACT_TABLE_LOAD runs right after the issues, hidden under the
            # input stream (gpsimd cannot read PSUM)
            scalar.wait_ge(pe_sem, 2)
            nc.scalar.copy(zt_sb[:, 1, :], pz[1][:, 0:NU]).then_inc(dveB)

        @block.tensor
        def _(tensor: bass.BassEngine):
            for sl in range(NSL):
                if sl % 4 == 0:
                    tensor.wait_ge(dxv[sl // 4], 16)
                um_ap = comb_sb[:, sl, EH:EH + NU // 2].bitcast(F8)
                for eg in range(EC):
                    mm = nc.tensor.matmul(pz[eg][:, 0:NU],
                                          comb_sb[:, sl, eg * 128:(eg + 1) * 128],
                                          um_ap,
                                          start=(sl == 0), stop=(sl == NSL - 1))
                    if sl == NSL - 1:
                        mm.then_inc(pe_sem)
            # step2: G.T restricted to the 24 used u-cols per j-chunk;
            # m-outer keeps the packed-bank psum groups sequential
            for m in range(JC):
                for eg in range(EC):
                    if m == 0:
                        tensor.wait_ge(dveB if eg == 1 else dveA,
                                       1 if eg < 2 else 2)
                        tensor.wait_ge(dwv, 16)
                    mm = nc.tensor.matmul(pgt[:, m * TW:(m + 1) * TW],
                                          wv_sb[:, eg, m * 128:(m + 1) * 128],
                                          zt_sb[:, eg, m * TW:(m + 1) * TW],
                                          start=(eg == 0), stop=(eg == EC - 1))
            mm.then_inc(pe_sem)

        @block.vector
        def _(vector: bass.BassEngine):
            vector.wait_ge(pe_sem, 1)
            nc.vector.tensor_copy(zt_sb[:, 0, :], pz[0][:, 0:NU]).then_inc(dveA)
            vector.wait_ge(pe_sem, 3)
            nc.vector.tensor_copy(zt_sb[:, 2, :], pz[2][:, 0:NU]).then_inc(dveA)
            vector.wait_ge(pe_sem, 4)
            # diagonal-block c-sum straight out of PSUM: col 24m + h on
            # partitions 0:64 (even c = 2m), col 24m + 12 + h on 64:128
            # (odd c = 2m+1); reduce over m (innermost, stride 24)
            row = JC * TW
            half0 = bass.AP(pgt_h, 0, [[row, 64], [1, H], [TW, JC]])
            half1 = bass.AP(pgt_h, 64 * row + H, [[row, 64], [1, H], [TW, JC]])
            nc.vector.tensor_reduce(osum[:, 0, :], half0,
                                    mybir.AxisListType.X, mybir.AluOpType.add)
            nc.vector.tensor_reduce(osum[:, 1, :], half1,
                                    mybir.AxisListType.X, mybir.AluOpType.add
                                    ).then_inc(dres)

    nc.compile()
    return nc


def _get_nc():
    if "nc" not in _CACHE:
        _CACHE["nc"] = _build_nc()
    return _CACHE["nc"]


def _in_maps(inputs):
    v = np.ascontiguousarray(np.asarray(inputs["value"], dtype=np.float32))
    wvT = np.ascontiguousarray(np.asarray(inputs["Wv"], np.float32).T)
    um8 = np.ascontiguousarray(
        _umask().astype(ml_dtypes.float8_e4m3)).view(np.uint8)
    maps = []
    for c in range(8):
        b, eh = c // 2, c % 2
        xv = np.ascontiguousarray(v[b][:, eh * EH:(eh + 1) * EH]
                                  ).astype(ml_dtypes.bfloat16)
        xum = np.ascontiguousarray(
            np.concatenate([xv.view(np.uint8), um8], axis=1)
        ).view(ml_dtypes.bfloat16)
        wvh = wvT[eh * EH:(eh + 1) * EH, :]
        # wv_d[3t + eg] = wvh[eg*128 + t]
        wvp = np.ascontiguousarray(
            wvh.reshape(EC, 128, E).transpose(1, 0, 2).reshape(EH, E)
        ).astype(ml_dtypes.bfloat16)
        maps.append({"xum": xum, "wv": wvp})
    return maps


def _assemble(results):
    out = np.empty((B, S, E), np.float32)
    for b in range(B):
        t = results[2 * b]["out"] + results[2 * b + 1]["out"]
        row = np.ascontiguousarray((t[:, 0:H] + t[:, H:]).T).reshape(1, E)
        out[b] = np.broadcast_to(row, (S, E))
    return out


def run(inputs, trace=False, **kw):
    """Run on hardware; returns (full_output, BassKernelResults)."""
    nc = _get_nc()
    res = run_bass_kernel_spmd(nc, _in_maps(inputs), core_ids=list(range(8)),
                               trace=trace, **kw)
    return _assemble(res.results), res


def kernel(**inputs) -> np.ndarray:
    out, _ = run(inputs)
    return out


# revision 49
# speedup vs baseline: 1.0126x; 1.0104x over previous
"""nn_Attention_19121194402320 on 8 TRN2 NeuronCores (raw Bass, bf16/fp8).

The reference module is

    k = (key @ Wk.T).reshape(B, H, S, D)       # RAW reshape
    q, v analogously
    attn = softmax(q @ k.T, axis=-1)
    out  = einsum('bnqk,bnvd->bnqd', attn, v)  # NOTE the 'k' vs 'v' labels
    out.transpose(0,2,1,3).reshape(B, S, E)

The second einsum's contraction labels differ ('k' in the first operand,
'v' in the second), so einsum sums each independently:

    out[b,n,q,d] = (sum_k attn[b,n,q,k]) * (sum_v v[b,n,v,d])
                 = sum_v v[b,n,v,d]          (softmax rows sum to 1)

i.e. the output is the per-head column-sum of the V projection broadcast
over every query position; query/key/Wq/Wk do not affect it (verified to
7e-7 against the jax reference).

Math: raw-reshape head h of Y = value@Wv.T is the contiguous flat chunk
Y[b].flat[h*65536:(h+1)*65536].reshape(1024, 64); chunk g = 12s + c maps
to Y[s, 64c:64c+64].  With U[s, c*12+h] the 0/1 indicator that chunk
12s+c belongs to head h:

    step1  Z.T[e, u]  = sum_s Xv[s, e] U[s, u]        (24 mm, xv stationary)
    step2  G.T[j, u]  = sum_e Wv.T[e, j] Z.T[e, u]    (18 mm, wv stationary)
    out[h*64+d]       = sum_c G.T[64c+d, c*12+h]

Sharding: core = (batch, e-half).  Each core contracts HALF the e
dimension (384 of 768): xv = value[b][:, eh*384:], wv = Wv.T[eh*384:,:],
and produces a partial [64, 24] tile; the host sums the two partials per
batch (the contraction is linear, so no cross-core comm).  This halves
the dominant Wv load vs a head-split (590KB vs 1.18MB/core).

Computing G TRANSPOSED puts the needed diagonal blocks at PSUM partition
bases 64c (multiples of 32), so the DVE reads them directly — no
SBUF->SBUF gather DMAs.  Step2 only computes the 24 u-columns per
j-chunk that the extraction uses (rhs = zt[:, eg, 24m:24m+24]), so its
18 matmuls run at the 60ns LDWEIGHTS-pipelined floor.  The c-sum is two
strided tensor_reduces (pgt cols 24m+h on partitions 0:64, 24m+12+h on
64:128 — a fused on-device add of the two raced the DVE pipeline, so
the host adds the 2x12 partial columns instead).  All 1024 output rows
are identical: host transposes + tiles the summed [64, 12] tile.

The mask is exact in fp8 (0/1 values; bf16 x fp8 matmul verified exact
on HW) and rides packed inside the xv rows (912B combined rows, read
back via a bitcast AP view), so inputs stream as just 3 large DMAs with
one contiguous 3.6-4.6KB descriptor per partition (ring descriptor
fetch is serial with the transfer, so descriptor count is stream time).
Total input 1.52MB/core vs 2.04MB for the head-split version.
All accumulation fp32 in PSUM; 4 PSUM banks (groups are sequential per
bank — interleaved groups in one bank break on HW, verified).

Engine plan:
  sync   : xv+mask slot 0-3 load (1 DMA); final [64,24] output store
  scalar : xv+mask slot 4-7 + wv loads; zt cast eg=1
           (ACT_TABLE_LOAD hides under the input stream)
  PE     : step1 Z.T (24 mm, 3 psum banks); step2 G.T (18 mm, 1 bank)
  DVE    : zt casts eg=0,2 (fp32->bf16), 2 strided reduces
"""

from contextlib import ExitStack

import ml_dtypes
import numpy as np

import concourse.bass as bass
from concourse import bacc, mybir
from concourse.bass_utils import run_bass_kernel_spmd

B, S, E, H, D = 4, 1024, 768, 12, 64
EH = E // 2          # e-columns per core (384)
EC = EH // 128       # 3 e-chunks per core
NSL = 8              # s-slots of 128 rows (full 1024)
JC = E // 128        # 6 j-chunks
NU = 144             # mask columns, index c*12 + h
TW = 24              # u-columns touched per j-chunk (2 c-parities x 12 heads)
FP = mybir.dt.float32
BF = mybir.dt.bfloat16
F8 = mybir.dt.float8e4

_CACHE = {}


def _umask() -> np.ndarray:
    """U[s, c*12+h] = 1 iff chunk 12*s+c belongs to head h."""
    U = np.zeros((S, NU), np.float32)
    for c in range(12):
        for h in range(H):
            lo = max(0, (1024 * h - c + 11) // 12)
            hi = min(S, (1024 * (h + 1) - c + 11) // 12)
            U[lo:hi, c * H + h] = 1.0
    return U


def _build_nc():
    # Bass.__init__ unconditionally emits 4 const-tile memsets (gpsimd) and a
    # full all-engine barrier before user code; this kernel uses neither,
    # so suppress them during construction to shave NEFF startup time.
    _memset = bass.BassGpSimd.memset
    _barrier = bass.Bass.all_engine_barrier
    bass.BassGpSimd.memset = lambda self, ap, c: None
    bass.Bass.all_engine_barrier = lambda self, **kw: None
    try:
        nc = bacc.Bacc("TRN2", target_bir_lowering=False, debug=False)
    finally:
        bass.BassGpSimd.memset = _memset
        bass.Bass.all_engine_barrier = _barrier

    # s-mapping: slot = 4k+r, partition t <-> s = 512k+4t+r (one
    # contiguous 3648B descriptor per partition per DMA).  Queue split is
    # lopsided (SP: slots 0-3; Act: slots 4-7 + wv): concurrent queues
    # share the 16 rings per-descriptor round-robin (~130GB/s each vs
    # ~310 solo), so the small SP stream drains early and Act then runs
    # alone; wv - whose post-arrival work (18 small matmuls) is shortest
    # - lands last, just before it is needed.
    # xum row s = [xv row s (384 bf16) | mask row U[s] (144 fp8, packed
    # into 72 bf16 slots)]: the mask rides inside the xv DMAs - zero
    # extra descriptors or issues; the matmul reads it via a bitcast view.
    xum_d = nc.dram_tensor("xum", [S, EH + NU // 2], BF,
                           kind="ExternalInput").ap()
    # wv rows host-permuted: wv_d[3t + eg] = Wv.T[eh*384 + eg*128 + t]
    wv_d = nc.dram_tensor("wv", [EH, E], BF, kind="ExternalInput").ap()
    # out_d[d, c_parity*12 + h]: partial sums; host adds parities + e-halves,
    # transposes to the 768-wide row and tiles it over all 1024 s positions.
    out_d = nc.dram_tensor("out", [64, 2 * H], FP, kind="ExternalOutput").ap()

    comb_sb = nc.alloc_sbuf_tensor("comb_sb", [128, NSL, EH + NU // 2],
                                   BF).ap()
    wv_sb = nc.alloc_sbuf_tensor("wv_sb", [128, EC, E], BF).ap()
    zt_sb = nc.alloc_sbuf_tensor("zt_sb", [128, EC, NU], BF).ap()
    osum = nc.alloc_sbuf_tensor("osum", [64, 2, H], FP).ap()

    with ExitStack() as ctx:
        pz = [ctx.enter_context(nc.psum_tensor(f"pz{i}", [128, NU], FP))
              for i in range(EC)]
        pgt_h = ctx.enter_context(nc.psum_tensor("pgt", [128, TW], FP))
        pgt = pgt_h.ap()
        dxv = [ctx.enter_context(nc.semaphore(f"dxv{i}")) for i in range(2)]
        dwv = ctx.enter_context(nc.semaphore("dwv"))
        dres = ctx.enter_context(nc.semaphore("dres"))
        dout = ctx.enter_context(nc.semaphore("dout"))
        pe_sem = ctx.enter_context(nc.semaphore("pe_sem"))
        dveA = ctx.enter_context(nc.semaphore("dveA"))
        dveB = ctx.enter_context(nc.semaphore("dveB"))
        block = ctx.enter_context(nc.Block())

        def xv_dma(eng, k):
            eng.dma_start(
                comb_sb[:, 4 * k:4 * k + 4, :],
                xum_d[512 * k:512 * (k + 1), :].rearrange(
                    "(t r) e -> t r e", t=128)
            ).then_inc(dxv[k], 16)

        @block.sync
        def _(sync: bass.BassEngine):
            xv_dma(sync, 0)
            sync.wait_ge(dres, 1)
            sync.dma_start(out_d, osum.rearrange("p a b -> p (a b)")
                           ).then_inc(dout, 16)
            sync.wait_ge(dout, 16)

        @block.scalar
        def _(scalar: bass.BassEngine):
            xv_dma(scalar, 1)
            scalar.dma_start(wv_sb, wv_d.rearrange("(t eg) j -> t eg j", t=128)
                             ).then_inc(dwv, 16)
            # eg=1 cast off the DVE's critical path; the implicit
            # ACT_TABLE_LOAD runs right after the issues, hidden under the
            # input stream (gpsimd cannot read PSUM)
            scalar.wait_ge(pe_sem, 2)
            nc.scalar.copy(zt_sb[:, 1, :], pz[1][:, 0:NU]).then_inc(dveB)

        @block.tensor
        def _(tensor: bass.BassEngine):
            for sl in range(NSL):
                if sl % 4 == 0:
                    tensor.wait_ge(dxv[sl // 4], 16)
                um_ap = comb_sb[:, sl, EH:EH + NU // 2].bitcast(F8)
                for eg in range(EC):
                    mm = nc.tensor.matmul(pz[eg][:, 0:NU],
                                          comb_sb[:, sl, eg * 128:(eg + 1) * 128],
                                          um_ap,
                                          start=(sl == 0), stop=(sl == NSL - 1))
                    if sl == NSL - 1:
                        mm.then_inc(pe_sem)
            # step2: all 18 matmuls accumulate into ONE [128, 24] PSUM
            # region (a single sequential group) - because chunk m's rhs
            # selects mask cols 24m:24m+24, the PSUM accumulation itself
            # performs the c-sum: quadrant [0:64, 0:12] ends up holding
            # the even-c diagonal total and [64:128, 12:24] the odd-c
            # total (the other two quadrants are cross terms, ignored).
            # This replaces the strided tensor_reduces with 2 tiny copies.
            for m in range(JC):
                for eg in range(EC):
                    if m == 0:
                        tensor.wait_ge(dveB if eg == 1 else dveA,
                                       1 if eg < 2 else 2)
                        tensor.wait_ge(dwv, 16)
                    mm = nc.tensor.matmul(pgt[:, 0:TW],
                                          wv_sb[:, eg, m * 128:(m + 1) * 128],
                                          zt_sb[:, eg, m * TW:(m + 1) * TW],
                                          start=(m == 0 and eg == 0),
                                          stop=(m == JC - 1 and eg == EC - 1))
            mm.then_inc(pe_sem)

        @block.vector
        def _(vector: bass.BassEngine):
            vector.wait_ge(pe_sem, 1)
            nc.vector.tensor_copy(zt_sb[:, 0, :], pz[0][:, 0:NU]).then_inc(dveA)
            vector.wait_ge(pe_sem, 3)
            nc.vector.tensor_copy(zt_sb[:, 2, :], pz[2][:, 0:NU]).then_inc(dveA)
            vector.wait_ge(pe_sem, 4)
            nc.vector.tensor_copy(osum[:, 0, :], pgt[0:64, 0:H])
            nc.vector.tensor_copy(osum[:, 1, :], pgt[64:128, H:2 * H]
                                  ).then_inc(dres)

    nc.compile()
    return nc


def _get_nc():
    if "nc" not in _CACHE:
        _CACHE["nc"] = _build_nc()
    return _CACHE["nc"]


def _in_maps(inputs):
    v = np.ascontiguousarray(np.asarray(inputs["value"], dtype=np.float32))
    wvT = np.ascontiguousarray(np.asarray(inputs["Wv"], np.float32).T)
    um8 = np.ascontiguousarray(
        _umask().astype(ml_dtypes.float8_e4m3)).view(np.uint8)
    maps = []
    for c in range(8):
        b, eh = c // 2, c % 2
        xv = np.ascontiguousarray(v[b][:, eh * EH:(eh + 1) * EH]
                                  ).astype(ml_dtypes.bfloat16)
        xum = np.ascontiguousarray(
            np.concatenate([xv.view(np.uint8), um8], axis=1)
        ).view(ml_dtypes.bfloat16)
        wvh = wvT[eh * EH:(eh + 1) * EH, :]
        # wv_d[3t + eg] = wvh[eg*128 + t]
        wvp = np.ascontiguousarray(
            wvh.reshape(EC, 128, E).transpose(1, 0, 2).reshape(EH, E)
        ).astype(ml_dtypes.bfloat16)
        maps.append({"xum": xum, "wv": wvp})
    return maps


def _assemble(results):
    out = np.empty((B, S, E), np.float32)
    for b in range(B):
        t = results[2 * b]["out"] + results[2 * b + 1]["out"]
        row = np.ascontiguousarray((t[:, 0:H] + t[:, H:]).T).reshape(1, E)
        out[b] = np.broadcast_to(row, (S, E))
    return out


def run(inputs, trace=False, **kw):
    """Run on hardware; returns (full_output, BassKernelResults)."""
    nc = _get_nc()
    res = run_bass_kernel_spmd(nc, _in_maps(inputs), core_ids=list(range(8)),
                               trace=trace, **kw)
    return _assemble(res.results), res


def kernel(**inputs) -> np.ndarray:
    out, _ = run(inputs)
    return out
